# revision 20
# baseline (speedup 1.0000x reference)
"""Trainium2 Bass kernel for MultiHeadSelfAttention (B=4, S=2048, D=1024, H=16, hd=64, RoPE, causal).

Sharding: 8 cores = (batch b = c//2) x (head-group g = c%2, 8 heads each).
Each core computes its 8 heads' attention for its batch and a partial output
projection; host sums the two head-group partials per batch.

v2 schedule (PE is the bottleneck ~240us busy):
  - vproj prologue, then per head-block jb: attention(jb) with the q/k
    projection+RoPE units of jb+1 injected between score/exp groups so PE
    never idles while ACT runs the exps.
  - exps are computed 2 k-tiles wide ([128,1024]) to cut ACT fixed overhead.
  - PSUM->SBUF copies run on DVE (tensor_copy), keeping ACT for exps only.
  - phase-3 output chains for window w-1 are injected into attention(jb=3,w).
"""

import numpy as np

B, S, D = 4, 2048, 1024
H, HD = 16, 64
THETA = 10000.0
NCORES = 8

_cached = None


def _build_module(do_compile=True):
    import sys
    if "/opt/trn_rl_repo" not in sys.path:
        sys.path.insert(0, "/opt/trn_rl_repo")
    from contextlib import ExitStack
    import concourse.bacc as bacc
    import concourse.tile as tile
    from concourse import mybir
    import concourse.bass as bass_mod

    bf = mybir.dt.bfloat16
    f32 = mybir.dt.float32
    Copy = mybir.ActivationFunctionType.Copy
    Exp = mybir.ActivationFunctionType.Exp

    nc = bacc.Bacc("TRN2", target_bir_lowering=False, debug=False,
                   num_devices=NCORES)

    xT_d = nc.dram_tensor("xT", [D, S], bf, kind="ExternalInput")
    wqkvT_d = nc.dram_tensor("wqkvT", [D, 1536], bf, kind="ExternalInput")
    woT_d = nc.dram_tensor("woT", [512, D], bf, kind="ExternalInput")
    cosq_d = nc.dram_tensor("cosq", [128, S], bf, kind="ExternalInput")
    sinq_d = nc.dram_tensor("sinq", [128, S], bf, kind="ExternalInput")
    cosk_d = nc.dram_tensor("cosk", [128, S], bf, kind="ExternalInput")
    sink_d = nc.dram_tensor("sink", [128, S], bf, kind="ExternalInput")
    maskw_d = nc.dram_tensor("maskw", [2, 128, 1024], bf, kind="ExternalInput")
    pswap_d = nc.dram_tensor("pswap", [128, 128], bf, kind="ExternalInput")
    out_d = nc.dram_tensor("out_p", [S, D], bf, kind="ExternalOutput")

    NT = S // 128   # 16 token tiles
    NW = S // 512   # 4 q windows

    with tile.TileContext(nc) as tc, ExitStack() as ctx:
        const = ctx.enter_context(tc.tile_pool(name="const", bufs=1))
        work = ctx.enter_context(tc.tile_pool(name="work", bufs=2))

        # split big loads across both DMA queues so vproj can start early
        xT = []
        for i in range(8):
            t = const.tile([128, S], bf, name=f"xT{i}")
            eng = nc.sync if i % 2 == 0 else nc.gpsimd
            eng.dma_start(out=t, in_=xT_d[128 * i:128 * i + 128, :])
            xT.append(t)
        wq = []
        for i in range(8):
            t = const.tile([128, 1536], bf, name=f"wq{i}")
            eng = nc.sync if i % 2 == 0 else nc.gpsimd
            eng.dma_start(out=t[:, 1024:1536],
                          in_=wqkvT_d[128 * i:128 * i + 128, 1024:1536])
            wq.append(t)
        for i in range(8):
            eng = nc.sync if i % 2 == 1 else nc.gpsimd
            eng.dma_start(out=wq[i][:, 0:1024],
                          in_=wqkvT_d[128 * i:128 * i + 128, 0:1024])
        tabs = {}
        for nm, d in (("cosq", cosq_d), ("sinq", sinq_d),
                      ("cosk", cosk_d), ("sink", sink_d)):
            t = const.tile([128, S], bf, name=nm)
            nc.gpsimd.dma_start(out=t, in_=d[:, :])
            tabs[nm] = t
        pswap = const.tile([128, 128], bf, name="pswap")
        nc.gpsimd.dma_start(out=pswap, in_=pswap_d[:, :])
        maskw_sb = []
        for m in range(2):
            t = const.tile([128, 1024], bf, name=f"maskw{m}")
            nc.gpsimd.dma_start(out=t, in_=maskw_d[m, :, :])
            maskw_sb.append(t)
        woT = []
        for i in range(4):
            t = const.tile([128, D], bf, name=f"woT{i}")
            nc.gpsimd.dma_start(out=t, in_=woT_d[128 * i:128 * i + 128, :])
            woT.append(t)

        qT = [const.tile([128, S], bf, name=f"qT{j}") for j in range(4)]
        kT = [const.tile([128, S], bf, name=f"kT{j}") for j in range(4)]
        vaug = [const.tile([128, 8 * 65], bf, name=f"vaug{t}") for t in range(NT)]
        attnT = [const.tile([128, S], bf, name=f"attnT{j}") for j in range(4)]

        # ---------------- vproj prologue ----------------
        with tc.tile_pool(name="vp", bufs=2, space="PSUM") as vp:
            for tt in range(NT):
                psv = vp.tile([128, 512], f32, name="psv")
                for i in range(8):
                    nc.tensor.matmul(psv, lhsT=xT[i][:, 128 * tt:128 * tt + 128],
                                     rhs=wq[i][:, 1024:1536],
                                     start=(i == 0), stop=(i == 7))
                v3 = vaug[tt].rearrange("p (h c) -> p h c", c=65)
                nc.vector.tensor_copy(
                    v3[:, :, 0:64], psv.rearrange("p (h c) -> p h c", c=64))
                nc.vector.memset(v3[:, :, 64:65], 1.0)

        # ---------------- main: proj/RoPE interleaved with attention --------
        with tc.tile_pool(name="pp", bufs=2, space="PSUM") as pp, \
             tc.tile_pool(name="scps", bufs=2, space="PSUM") as scps, \
             tc.tile_pool(name="accps", bufs=1, space="PSUM") as accps, \
             tc.tile_pool(name="dscr", bufs=4, space="DRAM") as dscr:

            def emit_proj_unit(jb, u):
                """One (q|k, window) projection+RoPE unit: 9 PE matmuls + DVE."""
                dsti, w = divmod(u, NW)
                dst = qT[jb] if dsti == 0 else kT[jb]
                col0 = 512 * dsti
                cos_sb = tabs["cosq"] if dsti == 0 else tabs["cosk"]
                sin_sb = tabs["sinq"] if dsti == 0 else tabs["sink"]
                ws = slice(512 * w, 512 * w + 512)
                ps = pp.tile([128, 512], f32, name="pps")
                c0 = col0 + 128 * jb
                for i in range(8):
                    nc.tensor.matmul(ps, lhsT=wq[i][:, c0:c0 + 128],
                                     rhs=xT[i][:, ws],
                                     start=(i == 0), stop=(i == 7))
                raw = work.tile([128, 512], bf, name="raw")
                nc.vector.tensor_copy(raw, ps)
                sw_ps = pp.tile([128, 512], f32, name="pps")
                nc.tensor.matmul(sw_ps, lhsT=pswap, rhs=raw,
                                 start=True, stop=True)
                swp = work.tile([128, 512], bf, name="swp")
                nc.vector.tensor_copy(swp, sw_ps)
                t1 = work.tile([128, 512], bf, name="t1")
                nc.vector.tensor_mul(t1, raw, cos_sb[:, ws])
                t2 = work.tile([128, 512], bf, name="t2")
                nc.vector.tensor_mul(t2, swp, sin_sb[:, ws])
                nc.vector.tensor_add(dst[:, ws], t1, t2)

            def emit_ph3_chain(tt, jw, eng=None):
                ps_o = pp.tile([128, 512], f32, name="pps")
                for i in range(4):
                    nc.tensor.matmul(
                        ps_o,
                        lhsT=attnT[i][:, 128 * tt:128 * tt + 128],
                        rhs=woT[i][:, 512 * jw:512 * jw + 512],
                        start=(i == 0), stop=(i == 3))
                osb = work.tile([128, 512], bf, name="osb")
                nc.scalar.activation(osb, ps_o, Copy)
                (eng or nc.sync).dma_start(
                    out=out_d[128 * tt:128 * tt + 128,
                              512 * jw:512 * jw + 512],
                    in_=osb)

            for u in range(2 * NW):     # jb=0 proj up front
                emit_proj_unit(0, u)

            for jb in range(4):
                # fill-work generator for this attention block
                fill = [(emit_proj_unit, (jb + 1, u))
                        for u in range(2 * NW)] if jb < 3 else []
                fidx = 0
                cell = 0
                ncells = 2 * sum(2 * w + 2 for w in range(NW))   # 40

                for w in range(NW):
                    ws = slice(512 * w, 512 * w + 512)
                    accs = [accps.tile([65, 512], f32, name=f"acc{h2}")
                            for h2 in range(2)]
                    nkt = 4 * w + 4
                    ngr = 2 * w + 2
                    for g in range(ngr):
                        for h2 in range(2):
                            hs = slice(64 * h2, 64 * h2 + 64)
                            wide = scps.tile([128, 1024], f32, name="pss")
                            for i in range(2):
                                kt = 2 * g + i
                                nc.tensor.matmul(
                                    wide[:, 512 * i:512 * i + 512],
                                    lhsT=kT[jb][hs, 128 * kt:128 * kt + 128],
                                    rhs=qT[jb][hs, ws], start=True, stop=True)
                            ssb = work.tile([128, 1024], bf, name="ssb", bufs=4)
                            nc.scalar.activation(ssb, wide, Exp)
                            if g >= 2 * w:
                                nc.vector.tensor_mul(ssb, ssb,
                                                     maskw_sb[g - 2 * w])
                            h = 2 * jb + h2
                            for i in range(2):
                                kt = 2 * g + i
                                m = kt - 4 * w
                                c0 = 128 * m if m > 0 else 0
                                nc.tensor.matmul(
                                    accs[h2][:, c0:512],
                                    lhsT=vaug[kt][:, 65 * h:65 * h + 65],
                                    rhs=ssb[:, 512 * i + c0:512 * i + 512],
                                    start=(kt == 0), stop=(kt == nkt - 1))
                            # inject fill work to keep PE busy during exps
                            cell += 1
                            want = -(-cell * len(fill) // ncells)
                            while fidx < min(want, len(fill)):
                                fn, args = fill[fidx]
                                fn(*args)
                                fidx += 1
                    if jb == 3 and w >= 1:
                        # previous window's output chains: PE work that overlaps
                        # this window's divisions on DVE/Pool
                        for t in range(4 * (w - 1), 4 * w):
                            for jw in range(2):
                                emit_ph3_chain(t, jw)
                    for h2 in range(2):
                        # copy acc to SBUF so the PSUM bank frees immediately
                        accf = work.tile([65, 512], f32, name=f"accf{h2}")
                        nc.vector.tensor_copy(accf, accs[h2])
                        rdenf = work.tile([65, 512], f32, name="rdenf")
                        nc.vector.reciprocal(rdenf[64:65, :], accf[64:65, :])
                        scr = dscr.tile([1, 512], f32, name="scr")
                        nc.gpsimd.dma_start(out=scr, in_=rdenf[64:65, :])
                        bcast = work.tile([64, 512], f32, name="bcast")
                        b_ap = bass_mod.AP(tensor=scr.tensor, offset=scr.offset,
                                           ap=[[0, 64]] + [list(p) for p in scr.ap[1:]])
                        nc.gpsimd.dma_start(out=bcast, in_=b_ap)
                        if h2 == 0:
                            nc.vector.tensor_mul(attnT[jb][0:64, ws],
                                                 accf[0:64, :], bcast)
                        else:
                            atmp = work.tile([64, 512], bf, name="atmp")
                            nc.vector.tensor_mul(atmp, accf[0:64, :], bcast)
                            nc.gpsimd.dma_start(out=attnT[jb][64:128, ws], in_=atmp)
                while fidx < len(fill):
                    fn, args = fill[fidx]
                    fn(*args)
                    fidx += 1

            # phase-3 leftovers: last window of jb=3; split the final drain
            # across both DMA queues (division traffic is done by now)
            for tt in range(12, 16):
                for jw in range(2):
                    emit_ph3_chain(tt, jw, nc.sync if jw == 0 else nc.gpsimd)

    if do_compile:
        nc.compile()
    return nc


def _host_inputs(x, tp, Wqkv, Wo):
    import ml_dtypes
    bf16 = ml_dtypes.bfloat16

    inv = THETA ** (-np.arange(0, HD, 2, dtype=np.float64) / HD)      # [32]
    ang = inv[:, None] * tp.astype(np.float64)[None, :]               # [32, S]
    pidx = np.arange(128)
    fi = (pidx % 64) // 2
    sign = np.where(pidx % 2 == 0, -1.0, 1.0)
    cosk = np.cos(ang)[fi, :]
    sink = sign[:, None] * np.sin(ang)[fi, :]
    cosq = (cosk / 8.0).astype(bf16)
    sinq = (sink / 8.0).astype(bf16)
    cosk = cosk.astype(bf16)
    sink = sink.astype(bf16)

    maskw = np.zeros((2, 128, 1024), dtype=bf16)
    j = np.arange(512)[None, :]
    p = np.arange(128)[:, None]
    for m in range(4):
        maskw[m // 2, :, 512 * (m % 2):512 * (m % 2) + 512] = \
            (j >= 128 * m + p).astype(bf16)

    pswap = np.zeros((128, 128), dtype=bf16)
    pswap[np.arange(128), np.arange(128) ^ 1] = 1

    in_maps = []
    for c in range(NCORES):
        b, g = divmod(c, 2)
        wsel = np.concatenate([Wqkv[512 * g:512 * (g + 1)],
                               Wqkv[1024 + 512 * g:1024 + 512 * (g + 1)],
                               Wqkv[2048 + 512 * g:2048 + 512 * (g + 1)]], axis=0)
        in_maps.append({
            "xT": np.ascontiguousarray(x[b].T).astype(bf16),
            "wqkvT": np.ascontiguousarray(wsel.T).astype(bf16),
            "woT": np.ascontiguousarray(Wo[:, 512 * g:512 * (g + 1)].T).astype(bf16),
            "cosq": cosq, "sinq": sinq, "cosk": cosk, "sink": sink,
            "maskw": maskw, "pswap": pswap,
        })
    return in_maps


def kernel(**inputs):
    global _cached
    import sys
    if "/opt/trn_rl_repo" not in sys.path:
        sys.path.insert(0, "/opt/trn_rl_repo")
    from concourse import bass_utils

    x = np.asarray(inputs["x"], dtype=np.float32)
    tp = np.asarray(inputs["token_positions"])
    Wqkv = np.asarray(inputs["W_qkv"], dtype=np.float32)
    Wo = np.asarray(inputs["W_o"], dtype=np.float32)

    if _cached is None:
        _cached = _build_module()
    nc = _cached

    in_maps = _host_inputs(x, tp, Wqkv, Wo)
    res = bass_utils.run_bass_kernel_spmd(nc, in_maps, core_ids=list(range(NCORES)))

    out = np.empty((B, S, D), dtype=np.float32)
    for b in range(B):
        out[b] = (res.results[2 * b]["out_p"].astype(np.float32)
                  + res.results[2 * b + 1]["out_p"].astype(np.float32))
    return out
